# revision 48
# baseline (speedup 1.0000x reference)
"""Trainium2 Bass kernel for the non-local attention block (dense_transformer).

Reference computation per batch item b (x: [B=32, C=64, H=32, W=32], N=1024):
    xf    = x[b] reshaped [C, N]
    phi   = w_phi   @ xf                     [C, N]
    theta = (w_theta @ xf)^T                 [N, C]
    g     = (w_g @ xf)^T @ w_mv^T            [N, C]
    att   = theta @ phi                      [N, N]
    att   = att @ w_mk^T                     [N, N]
    att   = softmax(att, axis over rows n)
    out   = att @ g                          [N, C]
    final = w_mask @ out^T + xf              [C, N]

Key algebraic restructure: (theta @ phi) @ w_mk^T == theta @ (phi @ w_mk^T),
which removes the N^3 matmul (1073M MACs -> 2x67M MACs per batch).  The
softmax denominator divide is folded into the small g factor (64 wide).

Per-core layout (data-parallel, 4 batch items per core, processed as 2
stacked pairs occupying the 128 SBUF partitions; batch "b" on partitions
0-63, batch "c" on 64-127, PE quadrant tile-position packing runs both
batches' matmuls concurrently):
    T    = w_theta @ xf          [64, 1024]
    PhiT = xf^T @ w_phi^T        [1024, 64]
    GT   = xf^T @ (w_mv@w_g)^T   [1024, 64]
    P2   = PhiT^T @ w_mk^T       [64, 1024]  (accum over 8 m-chunks)
    S    = P2^T @ T              [1024, 1024] = att2^T
    E    = exp(S)  (ScalarE, fused row-sum via accum_out -> D)
    GTs  = GT * (1/D)            (fold softmax divide into g)
    O    = GTs^T @ E             [64, 1024]  (two n-half passes, accum over m)
    final= w_mask @ O + xf

Pipeline design (v2 — built from the v1 trace):
  * ScalarE exp is the hard floor (~45us busy per core); the kernel is
    organized so the ACT engine is continuously fed: psS pool has THREE
    [128,1024] slots so S production runs two exps ahead, and each chunk's
    8 S matmuls are emitted as one 4-quadrant-packed group.
  * The v1 head (21.6us to first exp) came from serial DMA issue plus all
    of stage-1/P2 being queued ahead of the first S chunk.  v2 issues x on
    the sync ring and wmk quarters on the GpSimd ring in parallel, warms
    the PE + preloads the exp ACT table immediately, and only PhiT/T/P2-q0
    gate the first exp (~6us).
  * PSUM (8 banks): psS 3x[128,1024] (6) | trans 2x[128,256] (1, P2
    quarters + mask pieces) | psO 1x[128,512] (1, O accumulates the two
    n-halves in sequential passes; h1 of pair p overlaps pair p+1).
  * Stage-1 (PhiT/T/GT) psums borrow psS slots as [128,1024] tiles (one
    cast each instead of many small-piece casts — DVE time matters).
  * The PE power/HAM duty cycle throttles sustained full-rate phases to
    K=4/8; per-chunk PE work (~3.5k cycles) is sized to roughly match the
    two-exp ACT window even at the throttled clock.

All matmul operands bf16; PSUM accumulation fp32; softmax sum in fp32 via
activation accum_out; residual add uses the bf16 x (adds ~0.3% rel err,
well within the 2e-2 budget).
"""

import numpy as np
import ml_dtypes

import concourse.bass as bass
import concourse.mybir as mybir
import concourse.tile as tile
from concourse.bass import _add_dep_helper
from concourse.bass_utils import run_bass_kernel_spmd

BF = mybir.dt.bfloat16
F32 = mybir.dt.float32
I16 = mybir.dt.int16
EXP = mybir.ActivationFunctionType.Exp

B, C, HH, WW = 32, 64, 32, 32
N = HH * WW          # 1024
NCORES = 8
BPC = B // NCORES    # 4 batch items per core
NPAIRS = BPC // 2    # 2 stacked pairs per core
NK = N // 128        # 8 chunks of 128 along the N dimension
NH = 512             # n-half

# T is pre-scaled by log2(e)*2^23 on the host, so S = att*log2e*2^23.
# ScalarE exps apply scale=ln2/2^23; DVE (Schraudolph) chunks build the
# fp32 bit pattern directly:  bf16_bits = round(S*2^-16 + BIAS*2^-16).
LOG2E23 = float(np.log2(np.e) * 2**23)
EXP_SCALE = float(np.log(2.0) / 2**23)
SCHRAU_BIAS = float((127 - 0.0579) * 2**23)
# chunk indices whose batch-c exp runs on the DVE (Schraudolph) per pair
# (not the last chunk — the DVE reduce would lengthen the tail's critical
# path)
DVE_KS = ()


def _build_body(nc, tc, consts, acts, bigacts, psS, psO,
                x16, wsmallT, wmkhT, out_e):
    lo = slice(0, 64)
    hi = slice(64, 128)

    def low():
        # bounded: ~5 chunks' worth of instructions later, NOT end-of-program
        # (a -100000 offset turns "filler" into "tail" — the scheduler only
        # runs it when nothing else is ready).
        return tc.high_priority(offset=-150)

    # ---- warmup: keep the PE busy from t=0 so the HAM K=8/8 flip lands
    # right as real work starts (~3.4us of activity needed).
    warm_in = consts.tile([128, 512], BF, tag="warm_in")
    nc.gpsimd.memset(warm_in[:], 0.0)
    warm_ps = psS.tile([128, N], F32, tag="psS", name="warm_ps")
    for i in range(12):
        nc.tensor.matmul(warm_ps[:, 0:256], lhsT=warm_in[:, 0:128],
                         rhs=warm_in[:, 0:256])

    # ---- input DMAs.  Two parallel HWDGE rings (~136 GB/s each): the sync
    # ring carries wsmall + x (gates stage-1); the scalar ring carries the
    # four wmk quarters (q0 gates P2-q0).  Nothing on the GpSimd SWDGE ring
    # — its dge_drain costs ~3.4us at kernel end.
    xr = x16.rearrange("(p q) n -> q p n", p=NPAIRS)
    xball = acts.tile([128, NPAIRS, N], BF, tag="xball", bufs=1)

    nc.sync.dma_start(xball[:, 0, :], xr[:, 0, :])

    wsmall = consts.tile([128, 4 * C], BF, tag="wsmall")
    nc.sync.dma_start(wsmall[:], wsmallT[:])
    wth = wsmall[:, 0 * C:1 * C]
    wph = wsmall[:, 1 * C:2 * C]
    wgv = wsmall[:, 2 * C:3 * C]
    wma = wsmall[:, 3 * C:4 * C]

    # wmk^T in k-quarter-major DRAM layout [4, 1024(m), 256(k)].  Quarter 0
    # is split across both rings (m-chunks 0-3 on sync behind x0, 4-7 first
    # on the scalar ring) so P2-q0 can start ~2us earlier.  The scalar-ring
    # DMA issues go BEFORE the ACT table preload (issues cost ~1us each on
    # the scalar queue; the preload only needs to beat the first real exp).
    wmk_q = []
    q0 = consts.tile([128, NK, 256], BF, tag="wmkq0")
    wmkr = wmkhT.rearrange("(j mc q) k -> j q mc k", j=4, mc=NK)
    nc.scalar.dma_start(q0[:, 4:8, :], wmkr[0, :, 4:8, :])
    nc.sync.dma_start(q0[:, 0:4, :], wmkr[0, :, 0:4, :])
    wmk_q.append(q0)
    for j in range(1, 4):
        t = consts.tile([128, NK, 256], BF, tag=f"wmkq{j}")
        nc.scalar.dma_start(t[:], wmkr[j])
        wmk_q.append(t)

    nc.sync.dma_start(xball[:, 1, :], xr[:, 1, :])

    # ---- ACT table preload: the ~2.7us table load + dummy exp, after the
    # scalar-ring DMA issues but well before the first real exp.
    preload_e = acts.tile([128, 32], BF, tag="preload_e", bufs=1)
    preload_d = acts.tile([128, 1], F32, tag="preload_d", bufs=1)
    nc.scalar.activation(preload_e[:], warm_in[:, 0:32], EXP,
                         accum_out=preload_d[:])

    st = [dict() for _ in range(NPAIRS)]

    def stage1_phit(p):
        """PhiT for pair p: [128(m-part), 64(c)] chunks; b in cols 0:512,
        c in cols 512:1024 of one [128,1024] psS-slot tile.  Copyback is
        split in m-halves so P2-q0 isn't gated on one long cast."""
        xb = xball[:, p, :]
        s = st[p]
        ps = psS.tile([128, N], F32, tag="psS", name="psPhiT")
        PhiT = acts.tile([128, N], BF, tag="PhiT", name="PhiT")
        for half in range(2):
            for m in range(half * 4, half * 4 + 4):
                mm = slice(m * 128, (m + 1) * 128)
                nc.tensor.matmul(ps[:, m * C:(m + 1) * C],
                                 lhsT=xb[lo, mm], rhs=wph[lo, :])
                nc.tensor.matmul(ps[:, NH + m * C:NH + (m + 1) * C],
                                 lhsT=xb[hi, mm], rhs=wph[hi, :])
            cc = slice(half * 256, half * 256 + 256)
            ncc = slice(NH + half * 256, NH + half * 256 + 256)
            nc.vector.tensor_copy(out=PhiT[:, cc], in_=ps[:, cc])
            nc.vector.tensor_copy(out=PhiT[:, ncc], in_=ps[:, ncc])
        s["PhiT"] = PhiT

    def stage1_t(p):
        xb = xball[:, p, :]
        s = st[p]
        ps = psS.tile([128, N], F32, tag="psS", name="psT")
        for h in range(2):
            hh = slice(h * NH, (h + 1) * NH)
            nc.tensor.matmul(ps[lo, hh], lhsT=wth[lo, :], rhs=xb[lo, hh])
            nc.tensor.matmul(ps[hi, hh], lhsT=wth[hi, :], rhs=xb[hi, hh])
        T_sb = acts.tile([128, N], BF, tag="T_sb", name="T_sb")
        nc.vector.tensor_copy(out=T_sb[:], in_=ps[:])
        s["T_sb"] = T_sb

    def stage1_gt(p):
        xb = xball[:, p, :]
        s = st[p]
        ps = psS.tile([128, N], F32, tag="psS", name="psGT")
        for m in range(NK):
            mm = slice(m * 128, (m + 1) * 128)
            nc.tensor.matmul(ps[:, m * C:(m + 1) * C],
                             lhsT=xb[lo, mm], rhs=wgv[lo, :])
            nc.tensor.matmul(ps[:, NH + m * C:NH + (m + 1) * C],
                             lhsT=xb[hi, mm], rhs=wgv[hi, :])
        GT = acts.tile([128, N], BF, tag="GT", name="GT")
        nc.vector.tensor_copy(out=GT[:], in_=ps[:])
        s["GT"] = GT
        s["GTs"] = acts.tile([128, N], BF, tag="GTs", name="GTs")

    def alloc_pair(p):
        s = st[p]
        s["P2"] = acts.tile([128, N], BF, tag="P2", name="P2")
        s["E_b"] = bigacts.tile([128, NK, N], BF, tag="E_b", name="E_b")
        s["E_c"] = bigacts.tile([128, NK, N], BF, tag="E_c", name="E_c")
        s["D"] = acts.tile([128, 2 * NK], F32, tag="D", name="D")
        s["R"] = acts.tile([128, 2 * NK], F32, tag="R", name="R")
        s["O_sb"] = acts.tile([128, N], BF, tag="O_sb", name="O_sb")

    def p2_quarter(p, j):
        """P2 column-quarter j (256 k's) for pair p, col-split by batch.
        Borrows a psS slot (uses only the first 256 columns)."""
        s = st[p]
        ps = psS.tile([128, N], F32, tag="psS", name="psP2")
        for m in range(NK):
            cc = slice(m * C, (m + 1) * C)
            ncc = slice(NH + m * C, NH + (m + 1) * C)
            nc.tensor.matmul(ps[lo, 0:256], lhsT=s["PhiT"][:, cc],
                             rhs=wmk_q[j][:, m, :],
                             start=(m == 0), stop=(m == NK - 1))
            nc.tensor.matmul(ps[hi, 0:256], lhsT=s["PhiT"][:, ncc],
                             rhs=wmk_q[j][:, m, :],
                             start=(m == 0), stop=(m == NK - 1))
        nc.vector.tensor_copy(out=s["P2"][:, j * 256:(j + 1) * 256],
                              in_=ps[:, 0:256])

    def s_group(p, k):
        """All 8 S matmuls for chunk k (both batches), 4-quadrant packed."""
        s = st[p]
        klo = slice(k * 128, k * 128 + 64)
        khi = slice(k * 128 + 64, (k + 1) * 128)
        sb = psS.tile([128, N], F32, tag="psS", name="psS_b")
        sc = psS.tile([128, N], F32, tag="psS", name="psS_c")
        last = None
        for h in range(2):
            hh = slice(h * NH, (h + 1) * NH)
            nc.tensor.matmul(sb[lo, hh], lhsT=s["P2"][lo, klo],
                             rhs=s["T_sb"][lo, hh])
            nc.tensor.matmul(sb[hi, hh], lhsT=s["P2"][lo, khi],
                             rhs=s["T_sb"][lo, hh])
            nc.tensor.matmul(sc[lo, hh], lhsT=s["P2"][hi, klo],
                             rhs=s["T_sb"][hi, hh])
            last = nc.tensor.matmul(sc[hi, hh], lhsT=s["P2"][hi, khi],
                                    rhs=s["T_sb"][hi, hh])
        return sb, sc, last

    def exp_chunk(p, k, sb, sc):
        s = st[p]
        nc.scalar.activation(s["E_b"][:, k, :], sb[:], EXP, scale=EXP_SCALE,
                             accum_out=s["D"][:, 2 * k:2 * k + 1])
        if k in DVE_KS:
            # Schraudolph on the Vector engine: bf16 bit pattern in one
            # tensor_scalar, row-sum via tensor_reduce.
            ec = s["E_c"][:, k, :]
            nc.vector.tensor_scalar(
                out=ec.bitcast(I16), in0=sc[:],
                scalar1=2.0**-16, scalar2=SCHRAU_BIAS * 2.0**-16,
                op0=mybir.AluOpType.mult, op1=mybir.AluOpType.add)
            nc.vector.tensor_reduce(
                out=s["D"][:, 2 * k + 1:2 * k + 2], in_=ec,
                axis=mybir.AxisListType.X, op=mybir.AluOpType.add)
        else:
            nc.scalar.activation(s["E_c"][:, k, :], sc[:], EXP,
                                 scale=EXP_SCALE,
                                 accum_out=s["D"][:, 2 * k + 1:2 * k + 2])

    def gts_chunk(p, k):
        s = st[p]
        cc = slice(k * C, (k + 1) * C)
        ncc = slice(NH + k * C, NH + (k + 1) * C)
        nc.vector.reciprocal(s["R"][:, 2 * k:2 * k + 2],
                             s["D"][:, 2 * k:2 * k + 2])
        nc.vector.tensor_scalar_mul(s["GTs"][:, cc], s["GT"][:, cc],
                                    s["R"][:, 2 * k:2 * k + 1])
        nc.vector.tensor_scalar_mul(s["GTs"][:, ncc], s["GT"][:, ncc],
                                    s["R"][:, 2 * k + 1:2 * k + 2])

    def o_pass_init(p, h):
        st[p][f"psO{h}"] = psO.tile([128, NH], F32, tag="psO",
                                    name=f"psO_p{p}h{h}")

    def o_chunk(p, m, h, after=None):
        """O accumulation chunk m for n-half h (both batches, col-split)."""
        s = st[p]
        cc = slice(m * C, (m + 1) * C)
        ncc = slice(NH + m * C, NH + (m + 1) * C)
        hh = slice(h * NH, (h + 1) * NH)
        ps = s[f"psO{h}"]
        mm1 = nc.tensor.matmul(ps[lo, :], lhsT=s["GTs"][:, cc],
                               rhs=s["E_b"][:, m, hh],
                               start=(m == 0), stop=(m == NK - 1))
        if after is not None:
            _add_dep_helper(mm1.ins, after.ins, reason="O after next S group")
        nc.tensor.matmul(ps[hi, :], lhsT=s["GTs"][:, ncc],
                         rhs=s["E_c"][:, m, hh],
                         start=(m == 0), stop=(m == NK - 1))

    def o_copyback(p, h):
        s = st[p]
        hh = slice(h * NH, (h + 1) * NH)
        nc.vector.tensor_copy(out=s["O_sb"][:, hh], in_=s[f"psO{h}"][:])

    def finish_half(p, t, out_sb=None):
        """mask conv + residual add + out DMA for n-half t of pair p."""
        s = st[p]
        if out_sb is None:
            out_sb = acts.tile([128, N], F32, tag="out_sb", name="out_sb")
            s["out_sb"] = out_sb
        nn = slice(t * NH, (t + 1) * NH)
        psM = psS.tile([128, N], F32, tag="psS", name="psM")
        nc.tensor.matmul(psM[lo, 0:NH], lhsT=wma[lo, :], rhs=s["O_sb"][lo, nn])
        nc.tensor.matmul(psM[hi, 0:NH], lhsT=wma[hi, :], rhs=s["O_sb"][hi, nn])
        nc.vector.tensor_tensor(out_sb[:, nn], psM[:, 0:NH],
                                xball[:, p, nn], mybir.AluOpType.add)
        nc.sync.dma_start(
            out_e[p * 128:(p + 1) * 128, t * NH:(t + 1) * NH],
            out_sb[:, nn])
        return out_sb

    def finish(p):
        ob = finish_half(p, 0)
        finish_half(p, 1, ob)

    # ================= emission schedule =================
    # head: pair 0's PhiT, T, P2-q0 gate the first exp; GT needed by O at k=1.
    stage1_phit(0)
    alloc_pair(0)
    p2_quarter(0, 0)
    stage1_t(0)
    stage1_gt(0)

    last = NPAIRS - 1
    # S-groups are emitted one window ahead of their exps, so in the
    # in-order PE queue the next chunk's S matmuls precede this window's
    # O-chunks and fillers (which would otherwise delay them ~0.7us/chunk).
    pend = {}
    for p in range(NPAIRS):
        nxt = p + 1
        o_pass_init(p, 0)
        o_pass_init(p, 1)
        if p == 0:
            pend[0] = s_group(0, 0)
        for k in range(NK):
            sb, sc, _ = pend.pop(k)
            exp_chunk(p, k, sb, sc)
            anchor = None
            if k < NK - 1:
                pend[k + 1] = s_group(p, k + 1)
                anchor = pend[k + 1][2]
            gts_chunk(p, k)
            if k >= 1:
                o_chunk(p, k - 1, 0, after=anchor)
                o_chunk(p, k - 1, 1)
            # ---- fillers, balanced across the chunk windows ----
            if k == 0:
                p2_quarter(p, 1)
                if p > 0:
                    o_chunk(p - 1, NK - 1, 0)
                    o_chunk(p - 1, NK - 1, 1)
                    o_copyback(p - 1, 0)
                    o_copyback(p - 1, 1)
            if k == 1 and p > 0:
                finish(p - 1)
            if k == 2:
                p2_quarter(p, 2)
            if k == 3:
                p2_quarter(p, 3)
            if nxt < NPAIRS:
                if k == 4:
                    stage1_phit(nxt)
                    alloc_pair(nxt)
                if k == 5:
                    stage1_t(nxt)
                if k == 6:
                    p2_quarter(nxt, 0)
                if k == 7:
                    stage1_gt(nxt)
                if k == NK - 1:
                    pend[0] = s_group(nxt, 0)

    # ---- tail: last pair's m=7 O chunks + finish, half-pipelined
    p = last
    o_chunk(p, NK - 1, 0)
    o_chunk(p, NK - 1, 1)
    o_copyback(p, 0)
    ob = finish_half(p, 0)
    o_copyback(p, 1)
    finish_half(p, 1, ob)


def _eliminate_redundant_waits(nc):
    """Transitive redundant-wait elimination over the final BIR stream.

    Tile's sem assignment is per-proc minimal but NOT transitively minimal:
    e.g. a matmul reusing a PSUM slot gets both (ACT >= k) [reader done] and
    (PE >= p) [previous writer done] waits, although observing ACT >= k
    already implies PE >= p (the reader waited on the writer).  The extra
    same-engine waits serialize the PE pipeline (no back-to-back streaming,
    no quadrant concurrency).

    Soundness relies on per-queue in-order completion (PE pc-monotone,
    ACT/DVE strict FIFO):  observing sem s >= v implies the v-th
    incrementing instruction and its whole same-queue prefix completed,
    hence all THEIR increments fired and all their waits were satisfied.
    """
    blocks = list(nc.m.functions[0].blocks)
    seq = []
    for blk in blocks:
        for ins in blk.instructions:
            seq.append(ins)

    def queue_key(ins):
        si = getattr(ins, "sync_info", None)
        nm = type(ins).__name__
        if nm in ("InstDMACopy", "InstTensorLoad", "InstTensorSave"):
            if si and si.on_update:
                return "Q" + si.on_update[0].ant_name
        return "E" + str(ins.engine)

    sem_count = {}
    incpoints = {}
    qpos = {}
    qidx = {}
    for ins in seq:
        qk = queue_key(ins)
        i = qpos.get(qk, 0)
        qidx[id(ins)] = (qk, i)
        qpos[qk] = i + 1
        si = getattr(ins, "sync_info", None)
        if si and si.on_update:
            for u in si.on_update:
                s = u.ant_name
                v = sem_count.get(s, 0) + (u.update_value or 1)
                sem_count[s] = v
                incpoints.setdefault(s, []).append((v, qk, i))

    per_queue = {}
    for ins in seq:
        qk, i = qidx[id(ins)]
        per_queue.setdefault(qk, []).append(ins)

    def merge(a, b):
        if not b:
            return a
        out = dict(a)
        for k, v in b.items():
            if out.get(k, 0) < v:
                out[k] = v
        return out

    comp_cache = {}

    def know_comp(qk, i):
        if i < 0:
            return {}
        key = (qk, i)
        if key in comp_cache:
            return comp_cache[key]
        know = dict(know_comp(qk, i - 1))
        ins = per_queue[qk][i]
        si = getattr(ins, "sync_info", None)
        if si:
            for w in (si.on_wait or []):
                if know.get(w.ant_name, 0) < w.wait_value:
                    know[w.ant_name] = w.wait_value
                    know = merge(know, know_from_obs(w.ant_name, w.wait_value))
        comp_cache[key] = know
        return know

    obs_cache = {}

    def _dma_sem(sem):
        return "DMA" in sem

    def know_from_obs(sem, v):
        if _dma_sem(sem):
            return {}
        key = (sem, v)
        if key in obs_cache:
            return obs_cache[key]
        obs_cache[key] = {}
        pts = incpoints.get(sem, [])
        know = {}
        if pts and all(q == pts[0][1] for _, q, _ in pts):
            for cnt, qk, i in pts:
                if cnt >= v:
                    if qk.startswith("E"):
                        know = dict(know_comp(qk, i))
                    know[sem] = cnt
                    break
        obs_cache[key] = know
        return know

    import os
    mode = os.environ.get("KERNEL_ELIM", "self")
    self_only = (mode == "self")

    def _same_queue_sem(sem, qk):
        pts = incpoints.get(sem, [])
        return bool(pts) and all(q == qk for _, q, _ in pts)

    dropped = 0
    kept = 0
    for qk, insts in per_queue.items():
        if not qk.startswith("E"):
            continue
        know = {}
        for ins in insts:
            si = getattr(ins, "sync_info", None)
            if not si:
                continue
            if type(ins).__name__ in ("InstDMACopy", "InstTensorLoad",
                                      "InstTensorSave", "InstTriggeredCopy"):
                continue
            waits = list(si.on_wait or [])
            if waits:
                changed = True
                waitset = waits[:]
                while changed:
                    changed = False
                    for w in waitset[:]:
                        if self_only and not _same_queue_sem(w.ant_name, qk):
                            continue
                        base = dict(know)
                        for w2 in waitset:
                            if w2 is w:
                                continue
                            base[w2.ant_name] = max(
                                base.get(w2.ant_name, 0), w2.wait_value)
                            base = merge(
                                base, know_from_obs(w2.ant_name, w2.wait_value))
                        if base.get(w.ant_name, 0) >= w.wait_value:
                            waitset.remove(w)
                            dropped += 1
                            changed = True
                            break
                for w in waitset:
                    kept += 1
                    know[w.ant_name] = max(know.get(w.ant_name, 0), w.wait_value)
                    know = merge(know, know_from_obs(w.ant_name, w.wait_value))
                if len(waitset) != len(waits):
                    ins.sync_info = mybir.SyncInfo(
                        on_wait=waitset, on_update=list(si.on_update or []))
    return dropped, kept


_SPLIT_WAIT_TYPES = {
    "InstMatmult", "InstTensorTensor", "InstTensorCopy", "InstActivation",
    "InstTensorScalarPtr", "InstTensorScalar", "InstReciprocal",
    "InstTensorReduce", "InstMemSet", "InstLdweights", "InstTranspose",
    "InstTensorTensorScan", "InstSelect", "InstCopy", "InstDMACopy",
    "InstTensorLoad", "InstTensorSave", "InstDrain",
}


def _split_matmul_waits(nc):
    """Walrus's TRN2 codegen allows at most one sync-wait per compute
    instruction.  Hoist every wait of a multi-wait instruction onto NoOps
    placed right before it on the same engine — the NX sequencer executes
    them in order, so semantics are identical.
    """
    cnt = 0
    for blk in nc.m.functions[0].blocks:
        insts = blk.instructions
        new = []
        for ins in insts:
            si = getattr(ins, "sync_info", None)
            if (type(ins).__name__ in _SPLIT_WAIT_TYPES and si is not None
                    and si.on_wait and len(si.on_wait) > 1):
                for j, w in enumerate(si.on_wait):
                    nop = mybir.InstNoOp(
                        name=f"{ins.name}-w{j}",
                        engine=ins.engine,
                        sync_info=mybir.SyncInfo(on_wait=[w], on_update=[]),
                        bass_nofuse=True,
                    )
                    new.append(nop)
                ins.sync_info = mybir.SyncInfo(
                    on_wait=[], on_update=list(si.on_update))
                cnt += 1
            new.append(ins)
        blk.instructions = new
    return cnt


def build_nc_full():
    nc = bass.Bass()
    # Per-core inputs.  x rows: pair p occupies partitions [0:128) as
    # (batch 2p on 0-63, batch 2p+1 on 64-127) after slicing.
    x16 = nc.declare_dram_parameter("x16", [BPC * C, N], BF, isOutput=False)
    wsmallT = nc.declare_dram_parameter("wsmallT", [128, 4 * C], BF,
                                        isOutput=False)
    wmkhT = nc.declare_dram_parameter("wmkhT", [4 * N, 256], BF,
                                      isOutput=False)
    out_e = nc.declare_dram_parameter("out", [BPC * C, N], F32, isOutput=True)

    with tile.TileContext(nc) as tc:
        with (
            tc.tile_pool(name="consts", bufs=1) as consts,
            tc.tile_pool(name="acts", bufs=2) as acts,
            tc.tile_pool(name="bigacts", bufs=2) as bigacts,
            tc.tile_pool(name="psS", bufs=3, space="PSUM") as psS,
            tc.tile_pool(name="psO", bufs=2, space="PSUM") as psO,
        ):
            _build_body(nc, tc, consts, acts, bigacts, psS, psO,
                        x16, wsmallT, wmkhT, out_e)
    import os
    if os.environ.get("KERNEL_ELIM", "1") != "0":
        d, k = _eliminate_redundant_waits(nc)
        print(f"wait elimination: dropped {d}, kept {k}")
    _split_matmul_waits(nc)
    return nc


def _prep_weights(w_phi, w_theta, w_g, w_mask, w_mv, w_mk):
    bf = ml_dtypes.bfloat16

    def dup(a):  # [64, 64] -> [128, 64], duplicated on both partition halves
        return np.ascontiguousarray(np.concatenate([a, a], axis=0)).astype(bf)

    w_gv = (w_mv.astype(np.float64) @ w_g.astype(np.float64)).astype(np.float32)
    w_theta_s = (w_theta.astype(np.float64) * LOG2E23).astype(np.float32)
    wsmall = np.concatenate(
        [dup(w_theta_s.T), dup(w_phi.T), dup(w_gv.T), dup(w_mask.T)], axis=1)
    # w_mk^T [m, k] -> k-quarter-major [4, m, 256] -> [4*m, 256]
    wmkT = np.ascontiguousarray(w_mk.T).astype(bf)
    wmkh = np.ascontiguousarray(
        wmkT.reshape(N, 4, 256).transpose(1, 0, 2)).reshape(4 * N, 256)
    return {
        "wsmallT": np.ascontiguousarray(wsmall),
        "wmkhT": wmkh,
    }


def kernel(x, w_phi, w_theta, w_g, w_mask, w_mv, w_mk, _trace=False):
    bf = ml_dtypes.bfloat16
    x = np.asarray(x, dtype=np.float32)
    weights = _prep_weights(np.asarray(w_phi, np.float32),
                            np.asarray(w_theta, np.float32),
                            np.asarray(w_g, np.float32),
                            np.asarray(w_mask, np.float32),
                            np.asarray(w_mv, np.float32),
                            np.asarray(w_mk, np.float32))

    xr = x.reshape(B, C, N)
    in_maps = []
    for i in range(NCORES):
        shard = np.ascontiguousarray(xr[i * BPC:(i + 1) * BPC]).reshape(BPC * C, N)
        m = {"x16": shard.astype(bf)}
        m.update(weights)
        in_maps.append(m)

    nc = build_nc_full()
    res = run_bass_kernel_spmd(nc, in_maps, list(range(NCORES)), trace=_trace)
    outs = [np.asarray(res.results[i]["out"]).reshape(BPC, C, HH, WW)
            for i in range(NCORES)]
    full = np.concatenate(outs, axis=0)
    if _trace:
        return full, res
    return full
